# revision 1
# baseline (speedup 1.0000x reference)
"""Trainium2 Bass kernel for nn_ExpectedKLDivergence.

Data-parallel over batch across 8 cores. The pairwise expected-KL term is
algebraically reduced (verified vs f64) to

    div[s] = P[s-1]*A[s] - c2*Q[s-1]*Q[s]          for 1 <= s < len
    A = p0*(ln p0 - c1) + p1*(ln p1 - c1),  P = p0+p1,  Q = p0-p1
    c1 = (ln b + ln(1-b))/2,  c2 = (ln b - ln(1-b))/2

The mask is a per-row prefix, so the host packs only the valid prefixes of
each row into a dense [128, W] stream per core (rows balanced across cores by
total valid length). A single eps=1e-6 separator element between rows makes
every cross-row pair term vanish through the multiplications (eps*ln(eps) ~
1e-28), so the device kernel needs no masking at all: the scalar engine takes
ln(x*e^-c1), the vector engine forms A/P/Q and the two shifted products, and
the tensor engine reduces them into PSUM with a ones-vector. The first-step
alpha-prior terms are computed from a tiny side input. Host combines the
per-core partials (exact c2 applied in f64) and divides by B.
"""

import numpy as np

import concourse.bacc as bacc
import concourse.mybir as mybir
import concourse.tile as tile
from concourse.bass_utils import run_bass_kernel_spmd

ALPHA = 0.1
BETA = 0.9
B, S = 512, 32768
NCORES = 8
P = 128                      # partitions
N = 2048                     # columns per tile
MM = 512                     # matmul free-dim chunk (one PSUM bank)
EPS = 1e-6                   # row separator / padding value (ln stays in a
                             # comfortable ACT range; joint terms ~1e-9 rel)

C1 = float((np.log(BETA) + np.log(1.0 - BETA)) / 2.0)
C2 = float((np.log(BETA) - np.log(1.0 - BETA)) / 2.0)
ESC = float(np.exp(-C1))     # Ln(x*ESC) = ln(x) - C1

OFFLOAD = 0                  # 0: all DVE; 1: P/Q adds on gpsimd; 2: +r2
_BUILT: dict = {}            # width -> compiled Bacc module


def _build(width: int, reps: int = 1, offload: int = 0, iob: int = 4, wkb: int = 2, lcb: int = 2, tn: int = 0):
    f32 = mybir.dt.float32
    bf = mybir.dt.bfloat16
    Ln = mybir.ActivationFunctionType.Ln
    add = mybir.AluOpType.add
    assert width % MM == 0
    TN = tn or N
    sizes = [TN] * (width // TN)
    if width % TN:
        sizes.append(width % TN)
    NT = len(sizes)
    starts = [sum(sizes[:i]) for i in range(NT)]

    nc = bacc.Bacc()
    p0d = nc.dram_tensor("p0", [P, width + 2], f32, kind="ExternalInput")
    p1d = nc.dram_tensor("p1", [P, width + 2], f32, kind="ExternalInput")
    f0d = nc.dram_tensor("f0", [P, 2], f32, kind="ExternalInput")
    outd = nc.dram_tensor("acc", [P, 4], f32, kind="ExternalOutput")

    with tile.TileContext(nc) as tc:
        with (
            tc.tile_pool(name="io", bufs=iob) as io,
            tc.tile_pool(name="lcp", bufs=lcb) as lcp,
            tc.tile_pool(name="wk", bufs=wkb) as wk,
            tc.tile_pool(name="cs", bufs=1) as cs,
            tc.tile_pool(name="psp", bufs=1, space="PSUM") as psp,
        ):
            ones = cs.tile([P, 1], bf, tag="ones")
            nc.gpsimd.memset(ones[:], 1.0)
            ps1 = psp.tile([1, MM], f32, tag="ps1")
            ps2 = psp.tile([1, MM], f32, tag="ps2")
            acc3 = cs.tile([P, 1], f32, tag="acc3")

            from contextlib import nullcontext
            loop_ctx = tc.For_i(0, reps, 1) if reps > 1 else nullcontext()
            with loop_ctx:
              for k in range(NT):
                NK = sizes[k]
                W = NK + 2
                x0 = io.tile([P, W], bf, tag="x0")
                nc.gpsimd.dma_start(x0[:], p0d[:, starts[k] : starts[k] + W])
                x1 = io.tile([P, W], bf, tag="x1")
                nc.gpsimd.dma_start(x1[:], p1d[:, starts[k] : starts[k] + W])

                lc0 = lcp.tile([P, W], bf, tag="lc0")
                nc.scalar.activation(lc0[:], x0[:], Ln, scale=ESC)
                lc1 = lcp.tile([P, W], bf, tag="lc1")
                nc.scalar.activation(lc1[:], x1[:], Ln, scale=ESC)

                af0 = wk.tile([P, W], bf, tag="af0")
                nc.vector.tensor_mul(af0[:], x0[:], lc0[:])
                af1 = wk.tile([P, W], bf, tag="af1")
                nc.vector.tensor_mul(af1[:], x1[:], lc1[:])
                a = wk.tile([P, W], bf, tag="a")
                nc.vector.tensor_add(a[:], af0[:], af1[:])

                ve_pq = nc.gpsimd if offload >= 1 else nc.vector
                pt = wk.tile([P, W], bf, tag="pt")
                ve_pq.tensor_add(pt[:], x0[:], x1[:])
                qt = wk.tile([P, W], bf, tag="qt")
                ve_pq.tensor_sub(qt[:], x0[:], x1[:])

                r1 = wk.tile([P, NK], bf, tag="af0")
                nc.vector.tensor_mul(r1[:], pt[:, 0:NK], a[:, 1 : NK + 1])
                r2 = wk.tile([P, NK], bf, tag="af1")
                ve_r2 = nc.gpsimd if offload >= 2 else nc.vector
                ve_r2.tensor_mul(r2[:], qt[:, 0:NK], qt[:, 1 : NK + 1])

                # partition+tile reduction on the tensor engine: every 512-col
                # chunk of r1/r2 accumulates into a single PSUM row
                for c in range(NK // MM):
                    nc.tensor.matmul(
                        ps1[:],
                        ones[:],
                        r1[:, c * MM : (c + 1) * MM],
                        start=(k == 0 and c == 0),
                        stop=(k == NT - 1 and c == NK // MM - 1),
                    )
                    nc.tensor.matmul(
                        ps2[:],
                        ones[:],
                        r2[:, c * MM : (c + 1) * MM],
                        start=(k == 0 and c == 0),
                        stop=(k == NT - 1 and c == NK // MM - 1),
                    )

            # first-timestep alpha-prior terms from the packed side input
            t0 = cs.tile([P, 2], bf, tag="t0")
            nc.gpsimd.dma_start(t0[:], f0d[:])
            l0 = cs.tile([P, 2], bf, tag="l0")
            nc.scalar.activation(l0[:, 0:1], t0[:, 0:1], Ln, scale=1.0 / (1.0 - ALPHA))
            nc.scalar.activation(l0[:, 1:2], t0[:, 1:2], Ln, scale=1.0 / ALPHA)
            e3 = cs.tile([P, 2], f32, tag="e3")
            nc.vector.tensor_mul(e3[:], t0[:], l0[:])
            nc.vector.tensor_reduce(acc3[:], e3[:], mybir.AxisListType.X, add)

            outsb = cs.tile([P, 4], f32, tag="outsb")
            nc.gpsimd.memset(outsb[:], 0.0)
            ps1_sb = cs.tile([1, MM], f32, tag="pss1")
            nc.vector.tensor_copy(ps1_sb[:], ps1[:])
            ps2_sb = cs.tile([1, MM], f32, tag="pss2")
            nc.vector.tensor_copy(ps2_sb[:], ps2[:])
            nc.vector.tensor_reduce(outsb[0:1, 0:1], ps1_sb[:], mybir.AxisListType.X, add)
            nc.vector.tensor_reduce(outsb[0:1, 1:2], ps2_sb[:], mybir.AxisListType.X, add)
            nc.vector.tensor_copy(outsb[:, 2:3], acc3[:])
            nc.sync.dma_start(outd[:], outsb[:])
    nc.compile()
    return nc


def _assign_rows(lengths):
    """Greedy LPT balance of rows across cores by packed size (len+1)."""
    order = np.argsort(-lengths)
    loads = np.zeros(NCORES, np.int64)
    rows = [[] for _ in range(NCORES)]
    for r in order:
        c = int(np.argmin(loads))
        rows[c].append(int(r))
        loads[c] += int(lengths[r]) + 1
    return rows, loads


def _prep_core(p0, p1, lengths, rows, width):
    """Pack valid prefixes of `rows` into [P, width+2] planes + alpha input.

    width = NT*N. Layout: col 0 is the halo (previous flat element), cols
    1..width hold the packed stream, last col is lookahead pad.
    """
    maps = {}
    for name, plane in (("p0", p0), ("p1", p1)):
        flat = np.full(P * width, EPS, np.float32)
        pos = 0
        for r in rows:
            L = int(lengths[r])
            flat[pos : pos + L] = plane[r, :L]
            pos += L + 1                      # eps separator
        arr = np.empty((P, width + 2), np.float32)
        arr[:, 1 : width + 1] = flat.reshape(P, width)
        arr[0, 0] = EPS                       # virtual past for first row
        arr[1:, 0] = arr[:-1, width]          # halo: previous flat element
        arr[:, width + 1] = EPS               # lookahead pad (never a current)
        maps[name] = arr
    f0 = np.empty((P, 2), np.float32)
    f0[:, 0] = 1.0 - ALPHA                    # pad rows contribute exactly 0
    f0[:, 1] = ALPHA
    nr = len(rows)
    f0[:nr, 0] = p0[rows, 0]
    f0[:nr, 1] = p1[rows, 0]
    maps["f0"] = f0
    return maps


def kernel(posterior, length):
    post = np.asarray(posterior, dtype=np.float32)
    ln = np.asarray(length).astype(np.int64)
    assert post.shape == (B, S, 2), post.shape
    lengths = np.clip(ln, 1, S)

    p0 = np.ascontiguousarray(post[..., 0])
    p1 = np.ascontiguousarray(post[..., 1])
    rows, loads = _assign_rows(lengths)
    # common packed width per partition, rounded up to MM granularity
    wmax = int(np.ceil(loads.max() / P))
    width = max(MM, -(-wmax // MM) * MM)

    in_maps = [
        _prep_core(p0, p1, lengths, rows[c], width) for c in range(NCORES)
    ]

    if width not in _BUILT:
        _BUILT[width] = _build(width, offload=OFFLOAD)
    res = run_bass_kernel_spmd(_BUILT[width], in_maps, core_ids=list(range(NCORES)))

    total = np.float64(0.0)
    for c, r in enumerate(res.results):
        acc = np.asarray(r["acc"], np.float64)
        total += acc[0, 0] - C2 * acc[0, 1] + acc[: len(rows[c]), 2].sum()
    return np.float32(total / B)



# revision 6
# speedup vs baseline: 1.3294x; 1.3294x over previous
"""Trainium2 Bass kernel for nn_ExpectedKLDivergence.

Data-parallel over batch across 8 cores. The pairwise expected-KL term is
algebraically reduced (verified vs f64) to

    total = first + T1 - C2*T2
    T1 = sum_s P[s-1]*A[s],   T2 = sum_s Q[s-1]*Q[s]
    A  = p0*(ln p0 - C1) + p1*(ln p1 - C1),  P = p0+p1,  Q = p0-p1
    C1 = (ln b + ln(1-b))/2,  C2 = (ln b - ln(1-b))/2

The ragged mask is a per-row prefix, so the host packs only the valid
prefixes of each row into a dense bf16 stream per core (rows balanced
across cores by total valid length), with eps separators making cross-row
pair terms vanish. Per tile the device loads one [128, 2*(NK+2)] bf16 slab
holding [x0 || x1]; the scalar engine computes ln(x*e^-C1) over the whole
slab, the vector engine forms af=x*lc, a=af0+af1, qt=x0-x1, and the tensor
engine does all shifted multiply+reduce work via a diagonal-accumulation
trick: for 128-col chunks, psum[128,128] += past_chunk.T @ curr_chunk, whose
diagonal accumulates sum_s past[s]*curr[s] (extracted once at the end with a
tensor_tensor_reduce against an identity matrix). This keeps every DVE
operand 4B-aligned (the +1 shift lives in the PE operand reads) and needs
only 3 DVE ops per slab. Host combines partials in f64 and divides by B.
"""

import numpy as np
import ml_dtypes

import concourse.bacc as bacc
import concourse.mybir as mybir
import concourse.tile as tile
from concourse.bass_utils import run_bass_kernel_spmd

ALPHA = 0.1
BETA = 0.9
B, S = 512, 32768
NCORES = 8
P = 128                      # partitions
TN = 4096                    # columns (stream positions) per tile
MMCH = 128                   # PE diagonal-trick chunk width
EPS = 1e-6                   # row separator / padding value

C1 = float((np.log(BETA) + np.log(1.0 - BETA)) / 2.0)
C2 = float((np.log(BETA) - np.log(1.0 - BETA)) / 2.0)
ESC = float(np.exp(-C1))     # Ln(x*ESC) = ln(x) - C1

BF = ml_dtypes.bfloat16
_BUILT: dict = {}            # width -> compiled Bacc module


def _tile_sizes(width: int):
    assert width % MMCH == 0
    sizes = [TN] * (width // TN)
    if width % TN:
        sizes.append(width % TN)
    return sizes


def _build(width: int, reps: int = 1):
    f32 = mybir.dt.float32
    bf = mybir.dt.bfloat16
    Ln = mybir.ActivationFunctionType.Ln
    add = mybir.AluOpType.add
    mult = mybir.AluOpType.mult
    sizes = _tile_sizes(width)
    NT = len(sizes)
    starts = [sum(sizes[:i]) for i in range(NT)]          # arr-col offsets
    cofs = [sum(2 * (s + 2) for s in sizes[:i]) for i in range(NT)]
    total_cols = sum(2 * (s + 2) for s in sizes)

    nc = bacc.Bacc()
    xd = nc.dram_tensor("xcat", [P, total_cols], bf, kind="ExternalInput")
    f0d = nc.dram_tensor("f0", [P, 2], bf, kind="ExternalInput")
    outd = nc.dram_tensor("acc", [P, 2 * MMCH + 4], f32, kind="ExternalOutput")

    with tile.TileContext(nc) as tc:
        with (
            tc.tile_pool(name="io", bufs=3) as io,
            tc.tile_pool(name="lcp", bufs=2) as lcp,
            tc.tile_pool(name="wk", bufs=2) as wk,
            tc.tile_pool(name="cs", bufs=1) as cs,
            tc.tile_pool(name="psp", bufs=1, space="PSUM") as psp,
        ):
            ps1 = psp.tile([P, MMCH], f32, tag="ps1")
            ps2 = psp.tile([P, MMCH], f32, tag="ps2")

            from contextlib import nullcontext
            loop_ctx = tc.For_i(0, reps, 1) if reps > 1 else nullcontext()
            with loop_ctx:
              for k in range(NT):
                NK = sizes[k]
                W1 = NK + 2                     # one plane's slab width
                WS = 2 * W1                     # [x0 || x1] slab width
                x = io.tile([P, WS], bf, tag="x")
                nc.sync.dma_start(x[:], xd[:, cofs[k] : cofs[k] + WS])

                lc = lcp.tile([P, WS], bf, tag="lc")
                nc.scalar.activation(lc[:], x[:], Ln, scale=ESC)

                af = wk.tile([P, WS], bf, tag="af")
                nc.vector.tensor_mul(af[:], x[:], lc[:])
                a = wk.tile([P, W1], bf, tag="a")
                nc.vector.tensor_add(a[:], af[:, 0:W1], af[:, W1:WS])
                qt = wk.tile([P, W1], bf, tag="qt")
                nc.vector.tensor_sub(qt[:], x[:, 0:W1], x[:, W1:WS])

                last_c = NK - MMCH
                for c in range(0, NK, MMCH):
                    first = k == 0 and c == 0
                    last = k == NT - 1 and c == last_c
                    nc.tensor.matmul(
                        ps1[:], x[:, c : c + MMCH], a[:, c + 1 : c + 1 + MMCH],
                        start=first, stop=False,
                    )
                    nc.tensor.matmul(
                        ps1[:], x[:, W1 + c : W1 + c + MMCH],
                        a[:, c + 1 : c + 1 + MMCH],
                        start=False, stop=last,
                    )
                    nc.tensor.matmul(
                        ps2[:], qt[:, c : c + MMCH], qt[:, c + 1 : c + 1 + MMCH],
                        start=first, stop=last,
                    )

            # epilogue (once per launch): PSUM copy-out + alpha terms; the
            # host takes the diagonal traces in f64
            outsb = cs.tile([P, 2 * MMCH + 4], f32, tag="outsb")
            nc.gpsimd.memset(outsb[:], 0.0)
            nc.vector.tensor_copy(outsb[:, 0:MMCH], ps1[:])
            nc.vector.tensor_copy(outsb[:, MMCH : 2 * MMCH], ps2[:])

            t0 = cs.tile([P, 2], bf, tag="t0")
            nc.sync.dma_start(t0[:], f0d[:])
            l0 = cs.tile([P, 2], bf, tag="l0")
            nc.scalar.activation(l0[:, 0:1], t0[:, 0:1], Ln, scale=1.0 / (1.0 - ALPHA))
            nc.scalar.activation(l0[:, 1:2], t0[:, 1:2], Ln, scale=1.0 / ALPHA)
            e3 = cs.tile([P, 2], f32, tag="e3")
            nc.vector.tensor_mul(e3[:], t0[:], l0[:])
            nc.vector.tensor_reduce(
                outsb[:, 2 * MMCH : 2 * MMCH + 1], e3[:], mybir.AxisListType.X, add
            )
            nc.sync.dma_start(outd[:], outsb[:])
    nc.compile()
    return nc


def _assign_rows(lengths):
    """Greedy LPT balance of rows across cores by packed size (len+1)."""
    order = np.argsort(-lengths)
    loads = np.zeros(NCORES, np.int64)
    rows = [[] for _ in range(NCORES)]
    for r in order:
        c = int(np.argmin(loads))
        rows[c].append(int(r))
        loads[c] += int(lengths[r]) + 1
    return rows, loads


def _prep_core(p0, p1, lengths, rows, width):
    """Pack valid prefixes of `rows` into one bf16 [x0||x1] slab stream.

    Per plane: col 0 is the halo (previous flat element), cols 1..width hold
    the packed stream, last col is lookahead pad. Tiles of the stream are
    emitted as concatenated [x0_tile || x1_tile] slabs (2 overlap cols).
    """
    arrs = []
    for plane in (p0, p1):
        flat = np.full(P * width, EPS, np.float32)
        pos = 0
        for r in rows:
            L = int(lengths[r])
            flat[pos : pos + L] = plane[r, :L]
            pos += L + 1                      # eps separator
        arr = np.empty((P, width + 2), np.float32)
        arr[:, 1 : width + 1] = flat.reshape(P, width)
        arr[0, 0] = EPS                       # virtual past for first row
        arr[1:, 0] = arr[:-1, width]          # halo: previous flat element
        arr[:, width + 1] = EPS               # lookahead pad (never a current)
        arrs.append(arr.astype(BF))
    sizes = _tile_sizes(width)
    total_cols = sum(2 * (s + 2) for s in sizes)
    xcat = np.empty((P, total_cols), BF)
    st = 0
    co = 0
    for NK in sizes:
        xcat[:, co : co + NK + 2] = arrs[0][:, st : st + NK + 2]
        xcat[:, co + NK + 2 : co + 2 * NK + 4] = arrs[1][:, st : st + NK + 2]
        st += NK
        co += 2 * NK + 4
    f0 = np.empty((P, 2), np.float32)
    f0[:, 0] = 1.0 - ALPHA                    # pad rows contribute exactly 0
    f0[:, 1] = ALPHA
    nr = len(rows)
    f0[:nr, 0] = p0[rows, 0]
    f0[:nr, 1] = p1[rows, 0]
    return {"xcat": xcat, "f0": f0.astype(BF)}


def kernel(posterior, length):
    post = np.asarray(posterior, dtype=np.float32)
    ln = np.asarray(length).astype(np.int64)
    assert post.shape == (B, S, 2), post.shape
    lengths = np.clip(ln, 1, S)

    p0 = np.ascontiguousarray(post[..., 0])
    p1 = np.ascontiguousarray(post[..., 1])
    rows, loads = _assign_rows(lengths)
    # common packed width per partition, rounded up to MMCH granularity
    wmax = int(np.ceil(loads.max() / P))
    width = max(MMCH, -(-wmax // MMCH) * MMCH)

    in_maps = [
        _prep_core(p0, p1, lengths, rows[c], width) for c in range(NCORES)
    ]

    if width not in _BUILT:
        _BUILT[width] = _build(width)
    res = run_bass_kernel_spmd(_BUILT[width], in_maps, core_ids=list(range(NCORES)))

    total = np.float64(0.0)
    for c, r in enumerate(res.results):
        acc = np.asarray(r["acc"], np.float64)
        t1 = np.trace(acc[:, 0:MMCH])
        t2 = np.trace(acc[:, MMCH : 2 * MMCH])
        total += t1 - C2 * t2 + acc[: len(rows[c]), 2 * MMCH].sum()
    return np.float32(total / B)
